# revision 13
# baseline (speedup 1.0000x reference)
"""Trainium2 Bass kernel for nn_EquivariantConvLayer_22076131902172.

Sharding: the B*N=2048 destination rows are distributed over 8 NeuronCores
(256 rows each, degree-sorted round-robin for load balance). Host (numpy) does
index-only prep: edge dedup, padded per-row neighbor tables, pos gathers,
feats packing, and folding of the (cg, mix_W) tensor product into one matrix.

Device per pair slot: geometry, unit vector, Gaussian-basis evaluation of the
radial-MLP scalar g(rn), 16 polynomial basis components, masked neighbor
reductions, and a folded tensor-product matmul P = (Z kron S') @ A in bf16.

Self-contained: no imports from sibling files.
"""
import numpy as np
import ml_dtypes

# ---------------------------------------------------------------------------
# problem constants (hardcoded from the problem spec)
# ---------------------------------------------------------------------------
L_MAX = 3
CH = {0: 32, 1: 16, 2: 8, 3: 8}
R_C = 5.0
N_RBF = 16
B, N, E = 2, 1024, 65536
EPS = 1e-8
NCORES = 8
RPC = (B * N) // NCORES          # rows per core = 256

NB = 48                          # gaussian basis size for radial fit
CAP = 7.0                        # rn clamp; g(rn) ~ exactly 0 beyond
SIG_MULT = 2.0
EXT = 0.25

BLK = [CH[l] * (2 * l + 1) for l in range(4)]
OFF = [0, 32, 80, 120]
CMTOT = 176
NF = 16                          # basis components (+ y2 channel = col 16)
NKC = 22                         # main K chunks (176*16/128)
NKY = 2                          # y2-channel K chunks (176 padded to 256)

BF16 = ml_dtypes.bfloat16


# ---------------------------------------------------------------------------
# host-side algorithm pieces (numpy only)
# ---------------------------------------------------------------------------
def _triples():
    for l1 in range(L_MAX + 1):
        for l2 in range(L_MAX + 1):
            for l3 in range(abs(l1 - l2), min(L_MAX, l1 + l2) + 1):
                yield l1, l2, l3


def _cg_np(l1, l2, l3):
    rng = np.random.RandomState(l1 * 100 + l2 * 10 + l3)
    return rng.normal(0.0, 0.2, size=(2 * l1 + 1, 2 * l2 + 1, 2 * l3 + 1)).astype(np.float32)


_BASIS = ['one', 'y', 'z', 'x', 'xy', 'yz', 'z2', 'xz', 'x2',
          'xyz', 'yz2', 'z3', 'xz2', 'x2y', 'x2z', 'x3']


def _build_M16():
    c1 = 0.4886025119029199
    c2a, c2b, c2c = 1.0925484305920792, 0.31539156525252005, 0.5462742152960396
    c3a, c3b, c3c, c3d, c3e = (0.5900435899266435, 2.890611442640554,
                               0.4570457994644658, 0.3731763325901154,
                               1.445305721320277)
    i = {n: k for k, n in enumerate(_BASIS)}
    M = np.zeros((16, NF), np.float64)
    My2 = np.zeros(16, np.float64)

    def setr(f, terms):
        for name, coef in terms:
            M[f, i[name]] += coef

    setr(0, [('one', 0.28209479177387814)])
    setr(1, [('y', c1)]); setr(2, [('z', c1)]); setr(3, [('x', c1)])
    setr(4, [('xy', c2a)])
    setr(5, [('yz', c2a)])
    setr(6, [('z2', 3 * c2b), ('one', -c2b)])
    setr(7, [('xz', c2a)])
    setr(8, [('x2', c2c)]); My2[8] = -c2c
    setr(9, [('x2y', 4 * c3a), ('yz2', c3a), ('y', -c3a)])
    setr(10, [('xyz', c3b)])
    setr(11, [('yz2', 5 * c3c), ('y', -c3c)])
    setr(12, [('z3', 5 * c3d), ('z', -3 * c3d)])
    setr(13, [('xz2', 5 * c3c), ('x', -c3c)])
    setr(14, [('x2z', 2 * c3e), ('z3', c3e), ('z', -c3e)])
    setr(15, [('x3', 4 * c3a), ('xz2', 3 * c3a), ('x', -3 * c3a)])
    return M.astype(np.float32), My2.astype(np.float32)


def _build_A(mix_W):
    A = np.zeros((CMTOT, 16, CMTOT), np.float64)
    foff = [0, 1, 4, 9]
    for l1, l2, l3 in _triples():
        cg = _cg_np(l1, l2, l3).astype(np.float64)
        W = np.asarray(mix_W[f'{l1}_{l2}_{l3}'], np.float64)
        m1n, f2n, k3n = cg.shape
        C1, C3 = CH[l1], CH[l3]
        blk = np.einsum('oc,mfk->cmfok', W, cg)
        A[OFF[l1]:OFF[l1] + C1 * m1n, foff[l2]:foff[l2] + f2n,
          OFF[l3]:OFF[l3] + C3 * k3n] += blk.reshape(C1 * m1n, f2n, C3 * k3n)
    M16, My2 = _build_M16()
    A2 = np.einsum('cfo,fp->cpo', A, M16.astype(np.float64))
    Ay2 = np.einsum('cfo,f->co', A, My2.astype(np.float64))
    return A2.astype(np.float32), Ay2.astype(np.float32)


def _fit_radial(rmlp):
    W1, b1 = np.asarray(rmlp['W1'], np.float64), np.asarray(rmlp['b1'], np.float64)
    W2, b2 = np.asarray(rmlp['W2'], np.float64), np.asarray(rmlp['b2'], np.float64)
    W3, b3 = np.asarray(rmlp['W3'], np.float64), np.asarray(rmlp['b3'], np.float64)

    def silu(x):
        return x / (1 + np.exp(-x))

    def g(rn):
        centers = np.linspace(0, R_C, N_RBF)
        sigma = R_C / N_RBF
        phi = np.exp(-(((rn[:, None] - centers) / sigma) ** 2))
        h = silu(phi @ W1.T + b1)
        h = silu(h @ W2.T + b2)
        return (h @ W3.T + b3).mean(-1)

    grid = np.linspace(0, CAP, 20001)
    gv = g(grid)
    c = np.linspace(-EXT, CAP + EXT, NB)
    s = SIG_MULT * (c[1] - c[0])
    Bm = np.exp(-(((grid[:, None] - c) / s) ** 2))
    Am = np.concatenate([Bm, np.ones((len(grid), 1))], 1)
    coef, *_ = np.linalg.lstsq(Am, gv, rcond=None)
    return ((c / s).astype(np.float32), np.float32(1.0 / s),
            coef[:NB].astype(np.float32), np.float32(coef[NB]))


def _prep_edges(edge_index):
    """Dedup edges -> (row ids, neighbor cols, per-row degree counts)."""
    eb, ei, ej = np.asarray(edge_index, np.int64)
    key = (eb * N + ei) * N + ej
    uniq = np.unique(key)
    rowg = (uniq // N).astype(np.int64)
    j = (uniq % N).astype(np.int64)
    counts = np.bincount(rowg, minlength=B * N)
    return rowg, j, counts


# ---------------------------------------------------------------------------
# walrus workarounds
# ---------------------------------------------------------------------------
def _patch_bass():
    """Two workarounds for this container's walrus build:
    1. multi-wait Drain at TileContext exit -> split onto single-wait NOPs;
    2. any instruction with >1 sync wait -> move extra waits onto single-wait
       EventSemaphore instructions inserted just before it (same engine)."""
    import concourse.tile as tile
    import concourse.bass as bass
    from concourse import mybir
    from concourse.vector_clock import ScopedClock
    import json as _json

    if not getattr(tile.TileContext, '_drain_patched', False):
        def _patched(self, tick_clock, wait_clock):
            nop_inst = self.nc.sync.nop(nofuse=True, hint="tile_exit_waits")
            wait_clock.add_sem_waits(
                nop_inst.ins, ScopedClock({None: tick_clock.global_clock}))
            si0 = nop_inst.ins.sync_info
            ow = list(si0.on_wait) if si0 is not None else []
            if len(ow) > 1:
                nop_inst.ins.sync_info = mybir.SyncInfo(
                    on_wait=ow[:1],
                    on_update=list(nop_inst.ins.sync_info.on_update))
                for w in ow[1:]:
                    n2 = self.nc.sync.nop(nofuse=True, hint="tile_exit_waits")
                    n2.ins.sync_info = mybir.SyncInfo(on_wait=[w], on_update=[])
            self.nc.sync.drain()
            self.nc.all_engine_barrier()
            assert self.sems is not None
            popped = self.nc._tile_sem_poison_stack.pop()
            assert popped is self._sem_poison
            self.nc.clear_and_free_semaphores(
                list(self.sems.allocated().values()))
            self.nc.all_engine_barrier()

        tile.TileContext._drain_and_barrier = _patched
        tile.TileContext._drain_patched = True

    if not getattr(bass.Bass, '_json_wait_split_patched', False):
        _orig_to_json_bytes = bass.Bass.to_json_bytes

        def _split_waits_json(self):
            raw = _orig_to_json_bytes(self)
            m = _json.loads(raw)
            changed = False
            for fn in m.get("functions", []):
                for bb in fn.get("blocks", []):
                    out = []
                    for ins in bb.get("instructions", []):
                        si = ins.get("sync_info")
                        if si and len(si.get("on_wait") or []) > 1:
                            waits = si["on_wait"]
                            for k, w in enumerate(waits[:-1]):
                                out.append({
                                    "debug": ins.get("debug", 0),
                                    "engine": ins["engine"],
                                    "ins": [], "outs": [],
                                    "name": f'{ins["name"]}_xw{k}',
                                    "opcode": "EventSemaphore",
                                    "sync_info": {"on_update": [],
                                                  "on_wait": [w]},
                                })
                            si["on_wait"] = [waits[-1]]
                            changed = True
                        out.append(ins)
                    bb["instructions"] = out
            return _json.dumps(m).encode() if changed else raw

        bass.Bass.to_json_bytes = _split_waits_json
        bass.Bass._json_wait_split_patched = True


# ---------------------------------------------------------------------------
# device kernel builder
# ---------------------------------------------------------------------------
def _build_device(KA, KB, inv_sig_f, c0_f, iters=1):
    """Bass program. Row tile A (partitions) has KA neighbor slots, row tile B
    has KB; slot free axis is [0:KA]=tile A, [KA:KA+KB]=tile B.
    iters>1 wraps the body in a For_i loop (timing experiments only)."""
    import concourse.bass as bass
    import concourse.tile as tile
    from concourse import mybir
    from concourse.masks import make_identity

    _patch_bass()

    P = 128
    TKT = KA + KB
    f32, bf = mybir.dt.float32, mybir.dt.bfloat16
    OP = mybir.AluOpType
    ACT = mybir.ActivationFunctionType
    NC17 = NF + 1                      # 16 basis comps + y2

    nc = bass.Bass(trn_type="TRN2")

    d_pj = nc.dram_tensor("pj", [P, 3 * TKT], f32, kind="ExternalInput")
    d_val = nc.dram_tensor("val", [P, TKT], f32, kind="ExternalInput")
    d_cst = nc.dram_tensor("cst", [P, 2 * NB + 6], f32, kind="ExternalInput")
    d_zx = nc.dram_tensor("zx", [P, NKC * RPC], f32, kind="ExternalInput")
    d_zy = nc.dram_tensor("zy", [P, NKY * RPC], f32, kind="ExternalInput")
    d_ad = nc.dram_tensor("ad", [P, NKC * CMTOT], f32, kind="ExternalInput")
    d_ay = nc.dram_tensor("ay", [P, NKY * CMTOT], f32, kind="ExternalInput")
    d_p0 = nc.dram_tensor("p0", [P, RPC], f32, kind="ExternalOutput")
    d_p1 = nc.dram_tensor("p1", [CMTOT - P, RPC], f32, kind="ExternalOutput")

    tsl = [slice(0, KA), slice(KA, TKT)]

    from contextlib import nullcontext
    with tile.TileContext(nc) as tc:
        with tc.tile_pool(name="sb", bufs=1) as pool, \
             tc.tile_pool(name="ps", bufs=1, space="PSUM") as pspool, \
             (tc.For_i(0, iters, 1) if iters > 1 else nullcontext()):

            # ---- input DMAs: one per DRAM tensor, spread over 3 queues
            CST = pool.tile([P, 2 * NB + 6], f32, name="cstt")
            nc.sync.dma_start(CST[:], d_cst[:])
            PJ = pool.tile([P, 3, TKT], f32, name="pjt")
            nc.sync.dma_start(PJ[:], d_pj[:].rearrange("p (d k) -> p d k", d=3))
            VAL = pool.tile([P, TKT], f32, name="valt")
            nc.sync.dma_start(VAL[:], d_val[:])
            ZXL = pool.tile([P, NKC, RPC], f32, name="zxl")
            nc.scalar.dma_start(ZXL[:], d_zx[:].rearrange("p (g r) -> p g r", g=NKC))
            ZYL = pool.tile([P, NKY, RPC], f32, name="zyl")
            nc.scalar.dma_start(ZYL[:], d_zy[:].rearrange("p (g r) -> p g r", g=NKY))
            ADL = pool.tile([P, NKC, CMTOT], f32, name="adl")
            nc.sync.dma_start(ADL[:], d_ad[:].rearrange("p (g r) -> p g r", g=NKC))
            AYL = pool.tile([P, NKY, CMTOT], f32, name="ayl")
            nc.sync.dma_start(AYL[:], d_ay[:].rearrange("p (g r) -> p g r", g=NKY))
            CS = CST[:, 0:NB]
            AL = CST[:, NB:2 * NB]
            PI = CST[:, 2 * NB:2 * NB + 6]
            ident = pool.tile([P, P], f32, name="ident")
            make_identity(nc, ident[:])

            # ---- geometry: rij = pos_i - pos_j
            R = [pool.tile([P, TKT], f32, name=f"r{d}") for d in range(3)]
            for d in range(3):
                for t in range(2):
                    nc.vector.tensor_scalar(
                        out=R[d][:, tsl[t]], in0=PJ[:, d, tsl[t]],
                        scalar1=PI[:, 2 * d + t:2 * d + t + 1], scalar2=-1.0,
                        op0=OP.subtract, op1=OP.mult)

            SQ = [pool.tile([P, TKT], f32, name=f"sqr{d}") for d in range(3)]
            for d in range(3):
                nc.scalar.activation(SQ[d][:], R[d][:], ACT.Square)
            RN2 = pool.tile([P, TKT], f32, name="rn2")
            nc.vector.tensor_add(RN2[:], SQ[0][:], SQ[1][:])
            nc.vector.tensor_add(RN2[:], RN2[:], SQ[2][:])
            RN = pool.tile([P, TKT], f32, name="rnt")
            nc.scalar.activation(RN[:], RN2[:], ACT.Sqrt)
            RNE = pool.tile([P, TKT], f32, name="rne")
            nc.vector.tensor_scalar(out=RNE[:], in0=RN[:], scalar1=float(EPS),
                                    scalar2=None, op0=OP.max)
            INV = pool.tile([P, TKT], f32, name="invt")
            nc.vector.reciprocal(INV[:], RNE[:])
            RS = pool.tile([P, TKT], f32, name="rst")
            nc.vector.tensor_scalar(out=RS[:], in0=RNE[:], scalar1=float(CAP),
                                    scalar2=float(inv_sig_f),
                                    op0=OP.min, op1=OP.mult)
            U = [pool.tile([P, TKT], f32, name=f"u{d}") for d in range(3)]
            for d in range(3):
                nc.vector.tensor_mul(U[d][:], R[d][:], INV[:])

            # ---- per-rowtile pipelined: radial -> w -> weighted basis ->
            # ---- reductions -> transpose-broadcast  (t=0 and t=1 overlap on
            # ---- different engines)
            gv = nc.vector
            gp = nc.gpsimd
            comp_idx = {n: k for k, n in enumerate(_BASIS)}
            comp_idx['y2'] = 16
            SBC = pool.tile([P, RPC], f32, name="sbc")
            YBC = pool.tile([P, RPC], f32, name="ybc")
            KKt = [KA, KB]
            for t in range(2):
                kk = KKt[t]
                sl = tsl[t]
                e0, e1 = (gv, gp) if t == 0 else (gp, gv)
                PRE = pool.tile([P, kk, NB], f32, tag=f"rad{t}", name=f"pre{t}")
                e0.tensor_tensor(
                    out=PRE[:],
                    in0=RS[:, sl, None].to_broadcast((P, kk, NB)),
                    in1=CS[:, None, :].to_broadcast((P, kk, NB)),
                    op=OP.subtract)
                GSQ = pool.tile([P, kk, NB], f32, tag=f"rad{t}b", name=f"gsq{t}")
                nc.scalar.activation(GSQ[:], PRE[:], ACT.Square)
                GEX = pool.tile([P, kk, NB], f32, tag=f"rad{t}", name=f"gex{t}")
                nc.scalar.activation(GEX[:], GSQ[:], ACT.Exp, scale=-1.0)
                GA = pool.tile([P, kk, NB], f32, tag=f"rad{t}b", name=f"ga{t}")
                e1.tensor_mul(GA[:], GEX[:],
                              AL[:, None, :].to_broadcast((P, kk, NB)))
                RSUM = pool.tile([P, kk], f32, tag=f"rsum{t}", name=f"rsum{t}")
                gv.tensor_reduce(RSUM[:], GA[:], axis=mybir.AxisListType.X,
                                 op=OP.add)
                W = pool.tile([P, kk], f32, tag=f"w{t}", name=f"wt{t}")
                e0.tensor_scalar(out=W[:], in0=RSUM[:], scalar1=float(c0_f),
                                 scalar2=None, op0=OP.add)
                e0.tensor_mul(W[:], W[:], VAL[:, sl])

                # w-weighted basis products into comp-major tile [P, 17, kk]
                CB = pool.tile([P, NF + 1, kk], f32, tag=f"cb{t}", name=f"cb{t}")

                def cslot(name):
                    return CB[:, comp_idx[name], :]

                def tt_into(name, a, b, eng):
                    o = cslot(name)
                    eng.tensor_mul(o, a, b)
                    return o

                ux, uy, uz = U[0][:, sl], U[1][:, sl], U[2][:, sl]
                e0.tensor_copy(cslot('one'), W[:])
                wx = tt_into('x', W[:], ux, e0)
                wy = tt_into('y', W[:], uy, e1)
                wz = tt_into('z', W[:], uz, e0)
                cxy = tt_into('xy', wx, uy, e1)
                cyz = tt_into('yz', wy, uz, e0)
                cz2 = tt_into('z2', wz, uz, e1)
                cxz = tt_into('xz', wx, uz, e0)
                cx2 = tt_into('x2', wx, ux, e1)
                tt_into('y2', wy, uy, e0)
                tt_into('xyz', cxy, uz, e1)
                tt_into('yz2', cyz, uz, e0)
                tt_into('z3', cz2, uz, e1)
                tt_into('xz2', cxz, uz, e0)
                tt_into('x2y', cxy, ux, e1)
                tt_into('x2z', cxz, ux, e0)
                tt_into('x3', cx2, ux, e1)

                SRM = pool.tile([P, NF + 1], f32, tag=f"srm{t}", name=f"srm{t}")
                gv.tensor_reduce(SRM[:], CB[:],
                                 axis=mybir.AxisListType.X, op=OP.add)

                srmx = pool.tile([P, P], f32, tag=f"srmx{t}", name=f"srmx{t}")
                e0.tensor_copy(
                    srmx[:].rearrange("p (f r) -> p f r", f=NF),
                    SRM[:, 0:NF, None].to_broadcast((P, NF, P // NF)))
                pst = pspool.tile([P, P], f32, tag=f"pst{t}", name=f"pst{t}",
                                  space="PSUM")
                nc.tensor.transpose(pst[:], srmx[:], ident[:])
                nc.scalar.copy(SBC[:, t * P:(t + 1) * P], pst[:])
                psy = pspool.tile([P, P], f32, tag=f"psy{t}", name=f"psy{t}",
                                  space="PSUM")
                nc.tensor.transpose(
                    psy[:], SRM[:, NF:NF + 1].to_broadcast((P, P)), ident[:])
                nc.scalar.copy(YBC[:, t * P:(t + 1) * P], psy[:])

            # ---- kron W chunks + P matmul accumulation (split per rowtile
            # ---- half so tile A's tail starts while tile B still computes)
            P0 = pspool.tile([P, RPC], f32, tag="p0", name="p0t", space="PSUM")
            P1 = pspool.tile([CMTOT - P, RPC], f32, tag="p1", name="p1t",
                             space="PSUM")
            WG = [pool.tile([P, RPC], f32, tag=f"wg{g}", name=f"wg{g}")
                  for g in range(NKC)]
            WYG = [pool.tile([P, RPC], f32, tag=f"wyg{k}", name=f"wyg{k}")
                   for k in range(NKY)]
            nmm = NKC + NKY
            for t in range(2):
                cs_ = slice(t * P, (t + 1) * P)
                for g in range(NKC):
                    eng = gv if (g + t) % 2 == 0 else gp
                    eng.tensor_mul(WG[g][:, cs_], ZXL[:, g, cs_], SBC[:, cs_])
                for k in range(NKY):
                    eng = gv if (k + t) % 2 == 0 else gp
                    eng.tensor_mul(WYG[k][:, cs_], ZYL[:, k, cs_], YBC[:, cs_])
                idx = 0
                for g in range(NKC):
                    nc.tensor.matmul(P0[:, cs_], ADL[:, g, 0:P], WG[g][:, cs_],
                                     start=(idx == 0), stop=(idx == nmm - 1))
                    nc.tensor.matmul(P1[:, cs_], ADL[:, g, P:CMTOT],
                                     WG[g][:, cs_],
                                     start=(idx == 0), stop=(idx == nmm - 1))
                    idx += 1
                for k in range(NKY):
                    nc.tensor.matmul(P0[:, cs_], AYL[:, k, 0:P], WYG[k][:, cs_],
                                     start=(idx == 0), stop=(idx == nmm - 1))
                    nc.tensor.matmul(P1[:, cs_], AYL[:, k, P:CMTOT],
                                     WYG[k][:, cs_],
                                     start=(idx == 0), stop=(idx == nmm - 1))
                    idx += 1

            OUT0 = pool.tile([P, RPC], f32, name="out0")
            OUT1 = pool.tile([CMTOT - P, RPC], f32, name="out1")
            nc.scalar.copy(OUT0[:], P0[:])
            nc.scalar.copy(OUT1[:], P1[:])
            nc.sync.dma_start(d_p0[:], OUT0[:])
            nc.sync.dma_start(d_p1[:], OUT1[:])

    return nc


# ---------------------------------------------------------------------------
# host packing / unpacking
# ---------------------------------------------------------------------------
def prepare(feats_l0, feats_l1, feats_l2, feats_l3, pos, edge_index, rmlp,
            mix_W):
    """Host prep -> (KA, KB, inv_sig, c0, in_maps, metas)."""
    feats = {0: feats_l0, 1: feats_l1, 2: feats_l2, 3: feats_l3}
    pos = np.asarray(pos, np.float32)
    rowg, jcol, counts = _prep_edges(edge_index)
    cs, inv_sig, alpha, c0 = _fit_radial(rmlp)
    A2, Ay2 = _build_A(mix_W)
    Z = np.concatenate(
        [np.asarray(feats[l], np.float32).reshape(B * N, BLK[l])
         for l in range(4)], 1)

    # degree-sorted round-robin row assignment for load balance
    order = np.argsort(-counts, kind='stable')        # dense rows first
    core_rows = [order[c::NCORES] for c in range(NCORES)]
    degA = max(int(counts[cr[:128]].max()) for cr in core_rows)
    degB = max(int(counts[cr[128:]].max()) for cr in core_rows)
    KA = max(8, -(-degA // 8) * 8)
    KB = max(8, -(-degB // 8) * 8)
    TKT = KA + KB

    KMX = max(KA, KB)
    nbr = np.tile((np.arange(B * N) % N)[:, None], (1, KMX)).astype(np.int64)
    valid = np.zeros((B * N, KMX), np.float32)
    starts = np.zeros(B * N + 1, np.int64)
    np.cumsum(counts, out=starts[1:])
    slot = np.arange(len(rowg)) - starts[rowg]
    nbr[rowg, slot] = jcol
    valid[rowg, slot] = 1.0

    posf = pos.reshape(B * N, 3)
    b_of_row = np.arange(B * N) // N
    qf = np.arange(128) // 8
    qc = np.arange(128) % 8

    A_dev = np.zeros((128, NKC, CMTOT), np.float32)
    for g in range(NKC):
        A_dev[:, g, :] = A2[8 * g + qc, qf, :]
    Ay_dev = np.zeros((128, NKY, CMTOT), np.float32)
    for k in range(NKY):
        cmi = 128 * k + np.arange(128)
        m = cmi < CMTOT
        Ay_dev[m, k, :] = Ay2[cmi[m], :]
    ad_flat = np.ascontiguousarray(A_dev.reshape(128, NKC * CMTOT))
    ay_flat = np.ascontiguousarray(Ay_dev.reshape(128, NKY * CMTOT))

    cst = np.zeros((128, 2 * NB + 6), np.float32)
    cst[:, 0:NB] = cs
    cst[:, NB:2 * NB] = alpha

    in_maps = []
    metas = []
    for core in range(NCORES):
        rows = core_rows[core]                        # [256] global row ids
        rA, rB = rows[:128], rows[128:]
        pj = np.zeros((128, 3, TKT), np.float32)
        vl = np.zeros((128, TKT), np.float32)
        for t, (rr, kk, off) in enumerate(((rA, KA, 0), (rB, KB, KA))):
            nb_t = nbr[rr][:, :kk]
            pj[:, :, off:off + kk] = pos[
                b_of_row[rr][:, None], nb_t].transpose(0, 2, 1)
            vl[:, off:off + kk] = valid[rr][:, :kk]
        cstc = cst.copy()
        cstc[:, 2 * NB:] = np.stack(
            [posf[rA, 0], posf[rB, 0], posf[rA, 1], posf[rB, 1],
             posf[rA, 2], posf[rB, 2]], 1)
        Zc = Z[rows]                                  # [256, 176]
        zx = np.empty((128, NKC, RPC), np.float32)
        for g in range(NKC):
            zx[:, g, :] = Zc[:, 8 * g + qc].T
        zy = np.zeros((128, NKY, RPC), np.float32)
        for k in range(NKY):
            cmi = 128 * k + np.arange(128)
            m = cmi < CMTOT
            zy[m, k, :] = Zc[:, cmi[m]].T
        in_maps.append({
            "pj": np.ascontiguousarray(pj.reshape(128, 3 * TKT)),
            "val": vl, "cst": cstc,
            "zx": np.ascontiguousarray(zx.reshape(128, NKC * RPC)),
            "zy": np.ascontiguousarray(zy.reshape(128, NKY * RPC)),
            "ad": ad_flat, "ay": ay_flat,
        })
        metas.append(rows)

    return KA, KB, float(inv_sig), float(c0), in_maps, metas


def postprocess(results, metas):
    P = np.zeros((B * N, CMTOT), np.float32)
    for core in range(NCORES):
        rows = metas[core]
        P[rows, 0:128] = results[core]["p0"].T
        P[rows, 128:CMTOT] = results[core]["p1"].T
    outs = []
    for l in range(4):
        blk = P[:, OFF[l]:OFF[l] + BLK[l]]
        outs.append(np.ascontiguousarray(
            blk.reshape(B, N, CH[l], 2 * l + 1)).astype(np.float32))
    return tuple(outs)


LAST_EXEC_NS = None
LAST_RESULTS = None


def kernel(feats_l0, feats_l1, feats_l2, feats_l3, pos, edge_index, rmlp,
           mix_W):
    from concourse.bass_utils import run_bass_kernel_spmd

    KA, KB, inv_sig, c0, in_maps, metas = prepare(
        feats_l0, feats_l1, feats_l2, feats_l3, pos, edge_index, rmlp, mix_W)
    nc = _build_device(KA, KB, inv_sig, c0)
    res = run_bass_kernel_spmd(nc, in_maps, core_ids=list(range(NCORES)))
    global LAST_EXEC_NS, LAST_RESULTS
    LAST_EXEC_NS = res.exec_time_ns
    LAST_RESULTS = res
    return postprocess(res.results, metas)
